# revision 2
# baseline (speedup 1.0000x reference)
"""Trainium2 Bass kernel for nn_ExchangeBlock v3 — SPMD across 8 NeuronCores.

v3 design:
- Edges permuted into parity classes (src&1, dst&1) per core so node-feature
  gathers read exact 256-elem bf16 rows via a class base offset into the
  pair-row table (no on-chip parity select). One gather pair per 2 blocks.
- Node rows host-packed bf16 [a(32) | b m-pad4 (64) | c m-pad6 (48) | 0...]
  so all outer products run at DVE 2x_1p (j-paired a, m-inner b/c).
  K = 2432 = 19 chunks.
- Geometry from host-streamed per-edge rows (no gather), 8 blocks per round;
  sqrt via exp(0.5*ln(d2)) on ScalarE (natural_log_exp table); RBF same set.
- TP: DVE outer products -> PE transposes (PSUM) -> ScalarE/DVE drains ->
  19 matmuls with stationary weight chunks, N=512, output psmixT[w, e].
- LN: psmixT -> ScalarE bf16 copy -> PE transpose back [e,w] -> bn_stats ->
  Newton rsqrt -> ScalarE ynorm -> PE transpose to ynormT.
- dfilter/MLP transposed (features on partitions); biases as per-partition
  activation bias columns; the ln_b*dfilter branch is folded into host
  weights (dfw2b @ mlp_w1) so it rides the pg2 matmuls on sactT; final dot
  is 4 accumulating M=1 matmuls; +b2 via ScalarE on the [1,512] row.
"""

import sys

sys.path.insert(0, "/opt/trn_rl_repo")

import numpy as np
import ml_dtypes

import concourse.bass as bass
import concourse.mybir as mybir
import concourse.tile as tile
from concourse import bacc
from concourse.bass_utils import run_bass_kernel_spmd
from concourse.masks import make_identity

F32 = mybir.dt.float32
BF16 = mybir.dt.bfloat16
I32 = mybir.dt.int32
I16 = mybir.dt.int16
AF = mybir.ActivationFunctionType
OP = mybir.AluOpType

L0, L1, L2 = 32, 16, 8
NS = 128
NB = 64
CUTOFF = 7.0
N_NODES = 50000
N_EDGES = 400000
NCORES = 8

BLK = 512
SUB = 4
P = 128
KTP = 2432            # 1024 (a, j-paired) + 1024 (b, m4) + 384 (c, m6)
NCHUNK = 19
GB = 8                # blocks per geometry round
RSQRT_MAGIC = 0x5F3759DF
NPAIR = N_NODES // 2
ROW = 256             # bf16 elems per node slot (512B)

E_CORE = N_EDGES // NCORES
_compiled = {}


def _patch_walrus_dge_levels():
    import concourse.bass_utils as _bu

    if getattr(_bu, "_dge_patched", False):
        return
    orig = _bu.run_command

    def patched(argv, **kw):
        if argv and "walrus_driver" in str(argv[0]) and not any(
            "dge-levels" in str(a) for a in argv
        ):
            argv = list(argv) + [
                "--dge-levels=io,spill_reload,scalar_dynamic_offset,"
                "vector_dynamic_offsets,dynamic_size,dst_reduce,transpose"
            ]
        return orig(argv, **kw)

    _bu.run_command = patched
    _bu._dge_patched = True


_patch_walrus_dge_levels()


def _patch_drain_and_barrier():
    """Hoist final-drain sync waits onto nops (SP Drain has no free slots)."""
    if getattr(tile.TileContext, "_dab_patched", False):
        return

    def patched(self, tick_clock, wait_clock):
        nc = self.nc
        nops = [nc.sync.nop() for _ in range(32)]
        drain_inst = nc.sync.drain()
        from concourse.tile import ScopedClock

        wait_clock.add_sem_waits(
            drain_inst.ins, ScopedClock({None: tick_clock.global_clock})
        )
        si = drain_inst.ins.sync_info
        waits = list(si.on_wait) if si and si.on_wait else []
        if waits:
            assert len(waits) <= len(nops), f"{len(waits)} waits > nop slots"
            si.on_wait = []
            for w, n in zip(waits, nops):
                n.ins.sync_info = mybir.SyncInfo(on_wait=[w], on_update=[])

        nc.all_engine_barrier()
        assert self.sems is not None
        popped = nc._tile_sem_poison_stack.pop()
        assert popped is self._sem_poison
        nc.clear_and_free_semaphores(list(self.sems.allocated().values()))
        nc.all_engine_barrier()

    tile.TileContext._drain_and_barrier = patched
    tile.TileContext._dab_patched = True


_patch_drain_and_barrier()


def _newton_rsqrt(nc, pool, u, n, magic_t, tag, iters=2):
    bits = pool.tile([P, n], I32, tag=f"{tag}_b")
    nc.vector.tensor_copy(out=bits[:].bitcast(F32), in_=u)
    nc.vector.tensor_scalar(
        out=bits[:], in0=bits[:], scalar1=1, scalar2=None,
        op0=OP.arith_shift_right,
    )
    yb = pool.tile([P, n], I32, tag=f"{tag}_y")
    nc.vector.tensor_tensor(
        out=yb[:], in0=magic_t[:, 0:1].to_broadcast([P, n]), in1=bits[:],
        op=OP.subtract,
    )
    y = yb[:].bitcast(F32)
    t1 = pool.tile([P, n], F32, tag=f"{tag}_t1")
    for _ in range(iters):
        nc.vector.tensor_mul(t1[:], y, y)
        nc.vector.tensor_mul(t1[:], t1[:], u)
        nc.vector.tensor_scalar(
            out=t1[:], in0=t1[:], scalar1=-0.5, scalar2=1.5, op0=OP.mult, op1=OP.add,
        )
        nc.vector.tensor_mul(y, y, t1[:])
    return yb


def _build(nblocks: int, classes: tuple):
    import os
    stage = os.environ.get("K_STAGE", "full")
    nc = bacc.Bacc("TRN2", target_bir_lowering=False, debug=False)

    assert nblocks % GB == 0
    nrounds = nblocks // GB

    tbl = nc.dram_tensor("tbl", (NPAIR, 2 * ROW), BF16, kind="ExternalInput").ap()
    xw16 = nc.dram_tensor("xw16", (nblocks // 2, P, 128), I16, kind="ExternalInput").ap()
    geo = nc.dram_tensor("geo", (nblocks * BLK, 18), F32, kind="ExternalInput").ap()
    wflat = nc.dram_tensor("wflat", (KTP, NS), BF16, kind="ExternalInput").ap()
    dfw1 = nc.dram_tensor("dfw1", (NB, 128), BF16, kind="ExternalInput").ap()
    dfw2g = nc.dram_tensor("dfw2g", (128, 128), BF16, kind="ExternalInput").ap()
    mlpw1r = nc.dram_tensor("mlpw1r", (128, 4, 128), BF16, kind="ExternalInput").ap()
    wbr = nc.dram_tensor("wbr", (128, 4, 128), BF16, kind="ExternalInput").ap()
    w2cols = nc.dram_tensor("w2cols", (128, 4), BF16, kind="ExternalInput").ap()
    dfb1c = nc.dram_tensor("dfb1c", (128, 1), F32, kind="ExternalInput").ap()
    dfb2gc = nc.dram_tensor("dfb2gc", (128, 1), F32, kind="ExternalInput").ap()
    bias2c = nc.dram_tensor("bias2c", (128, 4), F32, kind="ExternalInput").ap()
    b2sc = nc.dram_tensor("b2sc", (1, 1), F32, kind="ExternalInput").ap()
    offs = nc.dram_tensor("offs", (1, NB), F32, kind="ExternalInput").ap()
    out = nc.dram_tensor("out", (nblocks * BLK,), F32, kind="ExternalOutput").ap()

    width = CUTOFF / (NB - 1)
    coeff = 0.5 / (width * width)
    sqc = float(np.sqrt(coeff))

    with tile.TileContext(nc) as tc:
        with (
            tc.tile_pool(name="const", bufs=1) as constp,
            tc.tile_pool(name="io", bufs=3) as iop,
            tc.tile_pool(name="geop", bufs=2) as geop,
            tc.tile_pool(name="gx", bufs=6) as gxp,
            tc.tile_pool(name="ptb", bufs=3) as ptbp,
            tc.tile_pool(name="pts", bufs=2) as ptsp,
            tc.tile_pool(name="work", bufs=2) as workp,
            tc.tile_pool(name="mlp", bufs=2) as mlpp,
            tc.tile_pool(name="ps_tr", bufs=2, space="PSUM") as ps_tr,
            tc.tile_pool(name="ps_mix", bufs=2, space="PSUM") as ps_mix,
            tc.tile_pool(name="ps_ln", bufs=2, space="PSUM") as ps_ln,
            tc.tile_pool(name="ps_df", bufs=1, space="PSUM") as ps_df,
            tc.tile_pool(name="ps_g", bufs=1, space="PSUM") as ps_g,
        ):
            # ---- resident constants ----
            identb = constp.tile([P, P], BF16)
            make_identity(nc, identb[:])
            magic_t = constp.tile([P, 1], I32)
            nc.vector.memset(magic_t[:], RSQRT_MAGIC)
            nhalfpi_t = constp.tile([P, 1], F32)
            nc.vector.memset(nhalfpi_t[:], float(-np.pi / 2))

            w_sb = constp.tile([P, NCHUNK, P], BF16)
            nc.sync.dma_start(out=w_sb[:], in_=wflat.rearrange("(c p) w -> p c w", p=P))
            dfw1_sb = constp.tile([NB, 128], BF16)
            nc.sync.dma_start(out=dfw1_sb[:], in_=dfw1)
            dfw2g_sb = constp.tile([128, 128], BF16)
            nc.sync.dma_start(out=dfw2g_sb[:], in_=dfw2g)
            mlpw1_sb = constp.tile([P, 4, 128], BF16)
            nc.sync.dma_start(out=mlpw1_sb[:], in_=mlpw1r)
            wb_sb = constp.tile([P, 4, 128], BF16)
            nc.sync.dma_start(out=wb_sb[:], in_=wbr)
            w2_sb = constp.tile([P, 4], BF16)
            nc.sync.dma_start(out=w2_sb[:], in_=w2cols)
            dfb1_sb = constp.tile([P, 1], F32)
            nc.sync.dma_start(out=dfb1_sb[:], in_=dfb1c)
            dfb2g_sb = constp.tile([P, 1], F32)
            nc.sync.dma_start(out=dfb2g_sb[:], in_=dfb2gc)
            bias2_sb = constp.tile([P, 4], F32)
            nc.sync.dma_start(out=bias2_sb[:], in_=bias2c)
            b2_sb = constp.tile([1, 1], F32)
            nc.sync.dma_start(out=b2_sb[:], in_=b2sc)
            offs_sb = constp.tile([P, NB], F32)
            nc.sync.dma_start(out=offs_sb[:], in_=offs.to_broadcast([P, NB]))

            rbf_store = constp.tile([P, nblocks, SUB, NB], BF16)
            dist_store = constp.tile([P, nrounds, GB * SUB], F32)
            env_store = constp.tile([P, nrounds, GB * SUB], F32)

            # =========== Phase A: geometry + RBF (ln/exp table) ===========
            NQ = GB * SUB
            for r in range(nrounds):
                g = iop.tile([P, NQ, 18], F32, tag="geo")
                nc.sync.dma_start(
                    out=g[:],
                    in_=geo[r * GB * BLK : (r + 1) * GB * BLK, :].rearrange(
                        "(q p) x -> p q x", p=P
                    ),
                )
                tvp = geop.tile([P, NQ, 3, 3], F32, tag="tvp")
                nc.vector.tensor_tensor(
                    out=tvp[:],
                    in0=g[:, :, 0:3].unsqueeze(3).to_broadcast([P, NQ, 3, 3]),
                    in1=g[:, :, 9:18].rearrange("p q (i j) -> p q i j", j=3),
                    op=OP.mult,
                )
                rv = geop.tile([P, NQ, 3], F32, tag="rv")
                nc.vector.reduce_sum(
                    out=rv[:], in_=tvp[:].transpose([0, 1, 3, 2]),
                    axis=mybir.AxisListType.X,
                )
                nc.vector.tensor_add(rv[:], rv[:], g[:, :, 6:9])
                nc.vector.tensor_sub(rv[:], rv[:], g[:, :, 3:6])
                rv2 = geop.tile([P, NQ, 3], F32, tag="rv2")
                nc.vector.tensor_mul(rv2[:], rv[:], rv[:])
                d2 = geop.tile([P, NQ], F32, tag="d2")
                nc.vector.reduce_sum(out=d2[:], in_=rv2[:], axis=mybir.AxisListType.X)
                nc.vector.tensor_scalar(
                    out=d2[:], in0=d2[:], scalar1=1e-12, scalar2=None, op0=OP.max,
                )
                lg = geop.tile([P, NQ], F32, tag="lg")
                nc.scalar.activation(lg[:], d2[:], AF.Ln)
                dist = dist_store[:, r, :]
                nc.scalar.activation(dist, lg[:], AF.Exp, scale=0.5)

                z = geop.tile([P, NQ, NB], F32, tag="z")
                nc.vector.tensor_tensor(
                    out=z[:],
                    in0=offs_sb[:].unsqueeze(1).to_broadcast([P, NQ, NB]),
                    in1=dist.unsqueeze(2).to_broadcast([P, NQ, NB]),
                    op=OP.subtract,
                )
                nc.scalar.activation(z[:], z[:], AF.Square, scale=sqc)
                nc.scalar.activation(
                    rbf_store[:, r * GB : (r + 1) * GB, :, :].rearrange(
                        "p b s n -> p (b s) n"
                    ),
                    z[:], AF.Exp, scale=-1.0,
                )

            if stage == "geo":
                for b in range(nblocks):
                    acc = workp.tile([P, SUB], F32, tag="acc")
                    r, o = b // GB, (b % GB) * SUB
                    nc.vector.tensor_copy(out=acc[:], in_=dist_store[:, r, o : o + SUB])
                    nc.sync.dma_start(
                        out=out[b * BLK : (b + 1) * BLK].rearrange("(s p) -> p s", p=P),
                        in_=acc[:],
                    )

            # =========== Phase B prelude: envelope (silu/sin table) ===========
            for r in range(nrounds if stage != "geo" else 0):
                dist = dist_store[:, r, :]
                dc = geop.tile([P, NQ], F32, tag="dc")
                nc.vector.tensor_scalar(
                    out=dc[:], in0=dist, scalar1=CUTOFF, scalar2=None, op0=OP.min,
                )
                cosd = geop.tile([P, NQ], F32, tag="cosd")
                nc.scalar.activation(
                    cosd[:], dc[:], AF.Sin,
                    bias=nhalfpi_t[:, 0:1], scale=float(np.pi / CUTOFF),
                )
                mask = geop.tile([P, NQ], F32, tag="mask")
                nc.vector.tensor_scalar(
                    out=mask[:], in0=dist, scalar1=CUTOFF, scalar2=None, op0=OP.is_lt,
                )
                env = geop.tile([P, NQ], F32, tag="env")
                nc.vector.tensor_scalar(
                    out=env[:], in0=cosd[:], scalar1=-0.5, scalar2=0.5,
                    op0=OP.mult, op1=OP.add,
                )
                nc.vector.tensor_mul(env_store[:, r, :], env[:], mask[:])

            # =========== Phase B main loop (2 blocks per gather) ===========
            # Software-pipelined: pass1(b+1) is emitted before pass2(b) so
            # the in-order DVE stream never head-of-line-blocks on the
            # cross-engine LN/dfilter chain.
            def pass1(b, half, xs, xd):
                psmixT = ps_mix.tile([P, BLK], F32, tag="psmixT")
                for s in range(SUB):
                    hs = half * SUB + s
                    x1 = xs[:, hs, :]
                    x2 = xd[:, hs, :]
                    ptb = ptbp.tile([P, KTP], BF16, tag="ptb")
                    nc.vector.tensor_tensor(
                        out=ptb[:, 0:1024].rearrange(
                            "p (u v j) -> p u v j", v=32, j=2
                        ),
                        in0=x1[:, 0:32].rearrange("p (u j) -> p u j", j=2)
                        .unsqueeze(2).to_broadcast([P, 16, 32, 2]),
                        in1=x2[:, 144:208].rearrange("p (v j) -> p v j", j=2)
                        .unsqueeze(1).to_broadcast([P, 16, 32, 2]),
                        op=OP.mult,
                    )
                    nc.vector.tensor_tensor(
                        out=ptb[:, 1024:2048].rearrange(
                            "p (u v m) -> p u v m", v=16, m=4
                        ),
                        in0=x1[:, 32:96].rearrange("p (u m) -> p u m", m=4)
                        .unsqueeze(2).to_broadcast([P, 16, 16, 4]),
                        in1=x2[:, 32:96].rearrange("p (v m) -> p v m", m=4)
                        .unsqueeze(1).to_broadcast([P, 16, 16, 4]),
                        op=OP.mult,
                    )
                    nc.vector.tensor_tensor(
                        out=ptb[:, 2048:2432].rearrange(
                            "p (u v m) -> p u v m", v=8, m=6
                        ),
                        in0=x1[:, 96:144].rearrange("p (u m) -> p u m", m=6)
                        .unsqueeze(2).to_broadcast([P, 8, 8, 6]),
                        in1=x2[:, 96:144].rearrange("p (v m) -> p v m", m=6)
                        .unsqueeze(1).to_broadcast([P, 8, 8, 6]),
                        op=OP.mult,
                    )
                    pts = ptsp.tile([P, NCHUNK, P], BF16, tag="pts")
                    for gi, lo in enumerate(range(0, NCHUNK, 8)):
                        hi = min(lo + 8, NCHUNK)
                        ptp = ps_tr.tile([P, 8, P], BF16, tag="ptp")
                        for j, c in enumerate(range(lo, hi)):
                            nc.tensor.transpose(
                                ptp[:, j, :], ptb[:, c * P : (c + 1) * P], identb[:]
                            )
                        if (s * 3 + gi) % 2 == 0:
                            nc.scalar.copy(pts[:, lo:hi, :], ptp[:, 0 : hi - lo, :])
                        else:
                            nc.vector.tensor_copy(
                                pts[:, lo:hi, :], ptp[:, 0 : hi - lo, :]
                            )
                    for c in range(NCHUNK):
                        nc.tensor.matmul(
                            psmixT[:, s * P : (s + 1) * P],
                            lhsT=w_sb[:, c, :], rhs=pts[:, c, :],
                            start=(c == 0), stop=(c == NCHUNK - 1),
                        )

                # ---- LN stats ----
                psmixT_sb = workp.tile([P, BLK], BF16, tag="pxsb")
                nc.scalar.copy(psmixT_sb[:], psmixT[:])
                pse8 = ps_ln.tile([P, 8, P], BF16, tag="lnt")
                pse = pse8[:, 0:SUB, :]
                for s in range(SUB):
                    nc.tensor.transpose(
                        pse[:, s, :], psmixT_sb[:, s * P : (s + 1) * P], identb[:]
                    )
                mvall = geop.tile([P, SUB, 2], F32, tag="mvall")
                for s in range(SUB):
                    stats = geop.tile([P, 6], F32, tag="stats")
                    nc.vector.bn_stats(out=stats[:], in_=pse[:, s, :])
                    nc.vector.bn_aggr(out=mvall[:, s, :], in_=stats[:])
                return pse, mvall

            def pass2(b, pse, mvall):
                r, qo = b // GB, (b % GB) * SUB
                muv = mvall[:, :, 0]
                varv0 = mvall[:, :, 1]
                varv = geop.tile([P, SUB], F32, tag="varv")
                nc.vector.tensor_scalar(
                    out=varv[:], in0=varv0, scalar1=1e-5, scalar2=None, op0=OP.add,
                )
                ryl = _newton_rsqrt(nc, geop, varv[:], SUB, magic_t, "lnr", iters=1)
                rstd = ryl[:].bitcast(F32)
                tb = geop.tile([P, SUB], F32, tag="tb")
                nc.vector.tensor_mul(tb[:], muv, rstd)
                nc.vector.tensor_scalar(
                    out=tb[:], in0=tb[:], scalar1=-1.0, scalar2=None, op0=OP.mult,
                )

                # ---- dfilter + MLP (transposed) ----
                yn8 = ps_ln.tile([P, 8, P], BF16, tag="lnt")
                ynT_ps = yn8[:, 0:SUB, :]
                dembT_ps = yn8[0:NB, SUB : 2 * SUB, :]
                sactT = mlpp.tile([P, SUB, P], BF16, tag="sactT")
                dfsg = mlpp.tile([P, SUB, P], BF16, tag="dfsg")
                rgm = mlpp.tile([P, SUB, P], BF16, tag="rgm")
                ph_ps = ps_df.tile([P, BLK], F32, tag="ph")
                demb = workp.tile([P, SUB, NB], BF16, tag="demb")
                nc.vector.tensor_tensor(
                    out=demb[:], in0=rbf_store[:, b, :, :],
                    in1=env_store[:, r, qo : qo + SUB]
                    .unsqueeze(2).to_broadcast([P, SUB, NB]),
                    op=OP.mult,
                )
                for s in range(SUB):
                    ynorm = workp.tile([P, P], BF16, tag="ynorm")
                    nc.scalar.activation(
                        ynorm[:], pse[:, s, :], AF.Identity,
                        bias=tb[:, s : s + 1], scale=rstd[:, s : s + 1],
                    )
                    nc.tensor.transpose(ynT_ps[:, s, :], ynorm[:], identb[:])
                    nc.tensor.transpose(dembT_ps[:, s, :], demb[:, s, :], identb[:])
                dembT = workp.tile([NB, SUB, P], BF16, tag="dembT")
                nc.scalar.copy(dembT[:], dembT_ps[:])
                nc.tensor.matmul(
                    ph_ps[:], lhsT=dfw1_sb[:],
                    rhs=dembT[:].rearrange("p s x -> p (s x)"),
                    start=True, stop=True,
                )
                nc.scalar.activation(
                    sactT[:].rearrange("p s x -> p (s x)"), ph_ps[:],
                    AF.Silu, bias=dfb1_sb[:, 0:1],
                )
                pdfg_ps = ps_g.tile([P, BLK], F32, tag="pg2")
                nc.tensor.matmul(
                    pdfg_ps[:], lhsT=dfw2g_sb[:],
                    rhs=sactT[:].rearrange("p s x -> p (s x)"),
                    start=True, stop=True,
                )
                nc.scalar.activation(
                    dfsg[:].rearrange("p s x -> p (s x)"), pdfg_ps[:],
                    AF.Identity, bias=dfb2g_sb[:, 0:1],
                )
                for s in range(SUB):
                    nc.vector.tensor_tensor(
                        out=rgm[:, s, :], in0=ynT_ps[:, s, :], in1=dfsg[:, s, :],
                        op=OP.mult,
                    )

                gact = mlpp.tile([P, 4, BLK], BF16, tag="gact")
                for j in range(4):
                    pg2 = ps_g.tile([P, BLK], F32, tag="pg2")
                    nc.tensor.matmul(
                        pg2[:], lhsT=mlpw1_sb[:, j, :],
                        rhs=rgm[:].rearrange("p s x -> p (s x)"),
                        start=True, stop=False,
                    )
                    nc.tensor.matmul(
                        pg2[:], lhsT=wb_sb[:, j, :],
                        rhs=sactT[:].rearrange("p s x -> p (s x)"),
                        start=False, stop=True,
                    )
                    nc.scalar.activation(
                        gact[:, j, :], pg2[:], AF.Silu, bias=bias2_sb[:, j : j + 1],
                    )
                out_ps = ps_df.tile([P, BLK], F32, tag="ph")
                for j in range(4):
                    nc.tensor.matmul(
                        out_ps[0:1, :], lhsT=w2_sb[:, j : j + 1],
                        rhs=gact[:, j, :], start=(j == 0), stop=(j == 3),
                    )
                out_sb = workp.tile([1, BLK], F32, tag="outsb")
                nc.scalar.activation(
                    out_sb[:], out_ps[0:1, :], AF.Identity, bias=b2_sb[:, 0:1],
                )
                nc.sync.dma_start(
                    out=out[b * BLK : (b + 1) * BLK].rearrange("(o e) -> o e", o=1),
                    in_=out_sb[:],
                )

            for bb in range(0, nblocks if stage != "geo" else 0, 2):
                p1, p2 = classes[bb]
                assert classes[bb + 1] == (p1, p2)
                xwt = iop.tile([P, 128], I16, tag="xw")
                nc.sync.dma_start(out=xwt[:], in_=xw16[bb // 2])
                xs = gxp.tile([P, 2 * SUB, ROW], BF16, tag="xs")
                xd = gxp.tile([P, 2 * SUB, ROW], BF16, tag="xd")
                nc.gpsimd.dma_gather(
                    out_ap=xs[:], in_ap=tbl[:, p1 * ROW : (p1 + 1) * ROW],
                    idxs_ap=xwt[:, 0:64],
                    num_idxs=2 * BLK, num_idxs_reg=2 * BLK, elem_size=ROW,
                    elem_step=2 * ROW, single_packet=False,
                )
                nc.gpsimd.dma_gather(
                    out_ap=xd[:], in_ap=tbl[:, p2 * ROW : (p2 + 1) * ROW],
                    idxs_ap=xwt[:, 64:128],
                    num_idxs=2 * BLK, num_idxs_reg=2 * BLK, elem_size=ROW,
                    elem_step=2 * ROW, single_packet=False,
                )

                for half in range(2):
                    b = bb + half
                    if stage == "gather":
                        acc = workp.tile([P, SUB], F32, tag="acc")
                        nc.vector.reduce_sum(
                            out=acc[:],
                            in_=xs[:, half * SUB : (half + 1) * SUB, 0:144],
                            axis=mybir.AxisListType.X,
                        )
                        nc.sync.dma_start(
                            out=out[b * BLK : (b + 1) * BLK].rearrange(
                                "(s p) -> p s", p=P
                            ),
                            in_=acc[:],
                        )
                        continue
                    pse, mvall = pass1(b, half, xs, xd)
                    if stage == "tp":
                        acc = workp.tile([P, SUB], F32, tag="acc")
                        nc.vector.tensor_copy(out=acc[:], in_=mvall[:, :, 0])
                        nc.sync.dma_start(
                            out=out[b * BLK : (b + 1) * BLK].rearrange(
                                "(s p) -> p s", p=P
                            ),
                            in_=acc[:],
                        )
                        continue
                    pass2(b, pse, mvall)

    nc.compile()
    return nc


def _pack_rows(nodes):
    """[N, 120] f32 -> [N, 256] bf16 rows:
    [a(32) | b m-pad4 (64) | c m-pad6 (48) | a j-duplicated (64) | 0]."""
    n = nodes.shape[0]
    rows = np.zeros((n, ROW), np.float32)
    rows[:, 0:32] = nodes[:, 0:32]
    b = nodes[:, 32:80].reshape(n, 16, 3)
    rows[:, 32:96] = np.concatenate(
        [b, np.zeros((n, 16, 1), np.float32)], axis=2
    ).reshape(n, 64)
    c = nodes[:, 80:120].reshape(n, 8, 5)
    rows[:, 96:144] = np.concatenate(
        [c, np.zeros((n, 8, 1), np.float32)], axis=2
    ).reshape(n, 48)
    rows[:, 144:208] = np.repeat(nodes[:, 0:32], 2, axis=1)
    return rows.astype(ml_dtypes.bfloat16)


def _wrap16(idx_block):
    """int array [n] (n%16==0) -> wrapped int16 layout [128, n//16]."""
    w = idx_block.astype(np.int16).reshape(-1, 16).T
    return np.tile(w, (8, 1))


def _plan_blocks(edge_index):
    """Per-class block counts (max over cores), even per class, total % GB."""
    nblk = [0, 0, 0, 0]
    for c in range(NCORES):
        lo, hi = c * E_CORE, (c + 1) * E_CORE
        src = edge_index[0, lo:hi]
        dst = edge_index[1, lo:hi]
        cls = (src & 1) * 2 + (dst & 1)
        for q in range(4):
            n = int((cls == q).sum())
            nblk[q] = max(nblk[q], (n + BLK - 1) // BLK)
    nblk = [n + (n % 2) for n in nblk]
    total = sum(nblk)
    pad = (-total) % GB
    nblk[3] += pad if pad % 2 == 0 else pad + GB
    while sum(nblk) % GB:
        nblk[3] += 2
    return nblk


def _prep(inputs):
    nodes = np.asarray(inputs["nodes"], np.float32)
    edge_index = np.asarray(inputs["edge_index"]).astype(np.int64)
    graph_batch = np.asarray(inputs["graph_batch"]).astype(np.int64)
    cell = np.asarray(inputs["cell"], np.float32).reshape(32, 9)
    edge_shift = np.asarray(inputs["edge_shift"], np.float32)
    pos = np.asarray(inputs["pos"], np.float32)

    tbl = _pack_rows(nodes).reshape(NPAIR, 2 * ROW)

    alpha = 1.0 / np.sqrt(float(L0 * L0 + L1 * L1 + L2 * L2))
    w0 = np.asarray(inputs["W0"], np.float32) * alpha
    w1 = np.asarray(inputs["W1"], np.float32) * (alpha / np.sqrt(3.0))
    w2_ = np.asarray(inputs["W2"], np.float32) * (alpha / np.sqrt(5.0))
    wflat = np.zeros((KTP, NS), np.float32)
    wa = wflat[0:1024].reshape(16, 32, 2, NS)
    for j in range(2):
        wa[:, :, j, :] = w0[j::2, :, :]
    wb = wflat[1024:2048].reshape(16, 16, 4, NS)
    wb[:, :, 0:3, :] = np.repeat(w1[:, :, None, :], 3, axis=2)
    wc = wflat[2048:2432].reshape(8, 8, 6, NS)
    wc[:, :, 0:5, :] = np.repeat(w2_[:, :, None, :], 5, axis=2)

    ln_g = np.asarray(inputs["ln_g"], np.float32)
    ln_b = np.asarray(inputs["ln_b"], np.float32)
    df_w2 = np.asarray(inputs["df_w2"], np.float32)
    df_b2 = np.asarray(inputs["df_b2"], np.float32)
    mlp_w1 = np.asarray(inputs["mlp_w1"], np.float32)
    mlp_b1 = np.asarray(inputs["mlp_b1"], np.float32)
    mlp_w2 = np.asarray(inputs["mlp_w2"], np.float32).reshape(512)
    mlp_b2 = np.asarray(inputs["mlp_b2"], np.float32).reshape(1)

    dfw2g = df_w2 * ln_g[None, :]
    dfw2b = df_w2 * ln_b[None, :]
    dfb2g = df_b2 * ln_g
    dfb2b = df_b2 * ln_b
    bias2 = mlp_b1 + mlp_w1.T @ dfb2b
    wbfold = dfw2b @ mlp_w1          # [128h, 512g]

    bf = lambda a: np.ascontiguousarray(a).astype(ml_dtypes.bfloat16)

    common = {
        "tbl": tbl,
        "wflat": bf(wflat),
        "dfw1": bf(np.asarray(inputs["df_w1"], np.float32)),
        "dfw2g": bf(dfw2g),
        "mlpw1r": bf(mlp_w1.reshape(128, 4, 128)),
        "wbr": bf(wbfold.reshape(128, 4, 128)),
        "w2cols": bf(np.ascontiguousarray(mlp_w2.reshape(4, 128).T)),
        "dfb1c": np.asarray(inputs["df_b1"], np.float32).reshape(128, 1),
        "dfb2gc": np.ascontiguousarray(dfb2g.reshape(128, 1)),
        "bias2c": np.ascontiguousarray(bias2.reshape(4, 128).T),
        "b2sc": mlp_b2.reshape(1, 1),
        "offs": np.linspace(0.0, CUTOFF, NB, dtype=np.float32)[None, :],
    }

    nblk_cls = _plan_blocks(edge_index)
    nblocks = sum(nblk_cls)
    classes = []
    for q in range(4):
        classes.extend([(q >> 1, q & 1)] * nblk_cls[q])
    classes = tuple(classes)

    bcell_all = cell[graph_batch[edge_index[0]]]
    possrc_all = pos[edge_index[0]]
    posdst_all = pos[edge_index[1]]

    in_maps = []
    perms = []
    for ci in range(NCORES):
        lo, hi = ci * E_CORE, (ci + 1) * E_CORE
        src_c = edge_index[0, lo:hi]
        dst_c = edge_index[1, lo:hi]
        cls = (src_c & 1) * 2 + (dst_c & 1)
        order = np.argsort(cls, kind="stable")

        slot_edge = np.full(nblocks * BLK, -1, np.int64)
        pos_slot = 0
        for q in range(4):
            sel = order[cls[order] == q]
            slot_edge[pos_slot : pos_slot + len(sel)] = lo + sel
            pos_slot += nblk_cls[q] * BLK
        valid = slot_edge >= 0
        e = np.where(valid, slot_edge, 0)
        src = edge_index[0, e].copy()
        dst = edge_index[1, e].copy()
        # dummies: parity-correct node ids (node 0 parity 0, node 1 parity 1)
        pvec1 = np.repeat([c[0] for c in classes], BLK).astype(np.int64)
        pvec2 = np.repeat([c[1] for c in classes], BLK).astype(np.int64)
        src[~valid] = pvec1[~valid]
        dst[~valid] = pvec2[~valid]

        geo_arr = np.zeros((nblocks * BLK, 18), np.float32)
        geo_arr[:, 0:3] = np.where(valid[:, None], edge_shift[e], 0)
        geo_arr[:, 3:6] = np.where(valid[:, None], possrc_all[e], 0)
        geo_arr[:, 6:9] = np.where(valid[:, None], posdst_all[e], 0)
        geo_arr[:, 9:18] = np.where(valid[:, None], bcell_all[e], 0)

        xw = np.zeros((nblocks // 2, P, 128), np.int16)
        for bp in range(nblocks // 2):
            sl = slice(bp * 2 * BLK, (bp + 1) * 2 * BLK)
            xw[bp, :, 0:64] = _wrap16(src[sl] >> 1)
            xw[bp, :, 64:128] = _wrap16(dst[sl] >> 1)

        m = dict(common)
        m["xw16"] = xw
        m["geo"] = geo_arr
        in_maps.append(m)
        perms.append(slot_edge)

    return in_maps, perms, nblocks, classes


def _get_compiled(nblocks, classes):
    key = (nblocks, classes)
    if key not in _compiled:
        _compiled[key] = _build(nblocks, classes)
    return _compiled[key]


def kernel(**inputs) -> np.ndarray:
    in_maps, perms, nblocks, classes = _prep(inputs)
    nc = _get_compiled(nblocks, classes)
    res = run_bass_kernel_spmd(nc, in_maps, core_ids=list(range(NCORES)))
    result = np.zeros((N_EDGES,), np.float32)
    for c in range(NCORES):
        o = np.asarray(res.results[c]["out"], np.float32)
        sel = perms[c] >= 0
        result[perms[c][sel]] = o[sel]
    return result.reshape(N_EDGES, 1).astype(np.float32)
